# revision 17
# baseline (speedup 1.0000x reference)
"""Trainium2 Bass kernel for Bottleneck_refine — sparse patch-slot kernel, v4.

Architecture recap (see v2/v3 docstrings): only ACTIVE (group, 16x16-patch)
units run the conv path; each device "slot" packs two group-patch halves
into the 128 partitions (64+64 channels, block-diagonal weights) over a
self-contained 18x18 window of the mask-pre-zeroed input xm = x*expand(m).
The residual add + final relu + scatter happen on host in f32 (the
reference's own "scatter-add into residual" epilogue). conv3 output ships
as int8 with a global scale (max-abs error <= half step = 0.15% of
max|out|).

v4 additions:
- Role-based packing: weights are per-core INPUT DATA, so each core's two
  weight sets can serve ANY two (group_lower, group_upper) combos. The 117
  active halves then fit 8 cores x 8 slots x 2 halves (role capacities
  4/4/4/4 per core; sum_g ceil(n_g/4) = 31 <= 32 roles), vs 9 slots with
  globally-shared weights.
- fp8 precision mode (deterministic rel err, measured before shipping):
  conv1 = 4 fp8 DoubleRow matmuls (w1 split hi+lo fp8, x single fp8),
  conv2 = 9 fp8 DoubleRow matmuls (w2 hi+lo, 18 (plane, tap) combos
  cross-paired so every DR pair has ascending window offsets), t1 fp8,
  t2 bf16, conv3 bf16. The bf16 mode (x hi/lo fp8 DR conv1, bf16 conv2)
  is kept as a fallback knob.
- out DMAs ride the otherwise-idle Pool SWDGE queue; finer tail batches;
  the last slot's int8 conversion splits across DVE+ACT so the drain is
  short.
"""

import numpy as np
import ml_dtypes

G = 4
C_IN = 1024
H = 128
W = 128
NCORES = 8
P = 16
GRID = 18
NPX = GRID * GRID   # 324
INT = P * P         # 256

FP8 = ml_dtypes.float8_e4m3
BF16 = ml_dtypes.bfloat16
OUT_RANGE = 2.0
OUT_SCALE = 127.0 / OUT_RANGE

PREC = 'fp8'        # 'fp8' | 'bf16'


def _batches(nslot):
    """(x batches, out batches) as tuples of slot tuples."""
    xs = [(s,) for s in range(nslot)]
    ob, s = [], 0
    sizes = [2, 2, 1, 1, 1, 1, 1, 1]
    for sz in sizes:
        if s >= nslot:
            break
        sz = min(sz, nslot - s)
        ob.append(tuple(range(s, s + sz)))
        s += sz
    while s < nslot:
        ob.append((s,))
        s += 1
    return tuple(xs), tuple(ob)


# ---------------------------------------------------------------------------
# planning: mask -> role assignment
# ---------------------------------------------------------------------------
def _plan(mask):
    """Returns (ns, roles, placements):
    ns = (ns0, ns1) slots per wset;
    roles[core][w][half] = group index or None;
    placements = list of (group, (r, c), core, slot, half)."""
    m = np.asarray(mask)[0]
    actives = [[(r, c) for r in range(8) for c in range(8) if m[g, r, c] > 0.5]
               for g in range(G)]
    counts = [len(a) for a in actives]
    total = sum(counts)

    def try_split(ns0, ns1):
        pool = []
        for h in range(NCORES):
            pool += [(h, 0, 0, ns0), (h, 0, 1, ns0),
                     (h, 1, 0, ns1), (h, 1, 1, ns1)]
        pool.sort(key=lambda r: -r[3])
        need = sorted(range(G), key=lambda g: -counts[g])
        used = {}
        taken = [False] * len(pool)
        for g in need:
            rem = counts[g]
            for i, (h, w, hf, cap) in enumerate(pool):
                if rem <= 0:
                    break
                if taken[i] or cap == 0:
                    continue
                taken[i] = True
                used[(h, w, hf)] = (g, min(cap, rem))
                rem -= cap
            if rem > 0:
                return None
        return used

    nslot0 = max(1, -(-total // (2 * NCORES)))
    for nslot in range(nslot0, nslot0 + 8):
        cands = sorted(range(nslot + 1),
                       key=lambda n0: abs(n0 - (nslot - n0)))
        for ns0 in cands:
            ns1 = nslot - ns0
            used = try_split(ns0, ns1)
            if used is not None:
                roles = [[[None, None], [None, None]] for _ in range(NCORES)]
                placements = []
                ptr = [0] * G
                for (h, w, hf), (g, cnt) in sorted(used.items()):
                    roles[h][w][hf] = g
                    base = 0 if w == 0 else ns0
                    for i in range(cnt):
                        placements.append(
                            (g, actives[g][ptr[g]], h, base + i, hf))
                        ptr[g] += 1
                assert ptr == counts, (ptr, counts)
                return (ns0, ns1), roles, placements
    raise RuntimeError('no feasible slot packing found')


def _expand_mask_np(mask):
    m = np.asarray(mask)[0]
    m1 = np.repeat(m, C_IN // G, axis=0)
    return np.repeat(np.repeat(m1, P, axis=1), P, axis=2)


# ---------------------------------------------------------------------------
# host packing
# ---------------------------------------------------------------------------
def _hilo(a):
    hi = a.astype(FP8)
    lo = (a - hi.astype(np.float32)).astype(FP8)
    return hi, lo


def _pack_weights(w1, w2, w3, roles):
    """Per-core full block-diagonal lhsT tiles.

    wf [NCORES, 128, 16, 128] fp8: conv1, index ((p*2+hl)*2+mm)*2+i.
    wc [NCORES, 128, 36, 128] fp8 (fp8 mode): conv2 (plane, tap) combos,
       index p*18 + plane*9 + tap (plane 0 = w_hi, 1 = w_lo).
    wb bf16: fp8 mode [NCORES, 128, 8, 128] conv3 (index p*4+j);
             bf16 mode [NCORES, 128, 26, 128] conv2 taps + conv3."""
    W1 = np.asarray(w1)[:, :, 0, 0]
    W2 = np.asarray(w2)
    W3 = np.asarray(w3)[:, :, 0, 0]
    nbp = 4 if PREC == 'fp8' else 13
    wf = np.zeros((NCORES, 128, 16, 128), np.float32)
    wc = np.zeros((NCORES, 128, 36 if PREC == 'fp8' else 1, 128), np.float32)
    wb = np.zeros((NCORES, 128, 2 * nbp, 128), np.float32)
    for h in range(NCORES):
        for p in range(2):
            for hf in range(2):
                g = roles[h][p][hf]
                if g is None:
                    continue
                hs = slice(64 * hf, 64 * hf + 64)
                W1g = W1[64 * g:64 * g + 64]
                W2g = W2[64 * g:64 * g + 64]
                W3g = W3[256 * g:256 * g + 256]
                for mm in range(2):
                    for i in range(2):
                        kt = 2 * mm + i
                        blk = W1g[:, 64 * kt:64 * kt + 64].T
                        for hl in range(2):
                            idx = ((p * 2 + hl) * 2 + mm) * 2 + i
                            wf[h, hs, idx, hs] = blk
                for t in range(9):
                    dy, dx = t // 3 - 1, t % 3 - 1
                    blk = W2g[:, :, dy + 1, dx + 1].T
                    if PREC == 'fp8':
                        wc[h, hs, p * 18 + t, hs] = blk   # staging; see below
                    else:
                        wb[h, hs, p * nbp + t, hs] = blk
                for j in range(4):
                    off = p * nbp + (9 if PREC == 'bf16' else 0) + j
                    wb[h, hs, off, hs] = W3g[64 * j:64 * j + 64].T
    wfq = np.zeros(wf.shape, FP8)
    for idx in range(16):
        hl = (idx >> 2) & 1
        hi, lo = _hilo(wf[:, :, idx, :])
        wfq[:, :, idx, :] = hi if hl == 0 else lo
    # conv2 fp8 tiles are laid out in DoubleRow PAIR order: index
    # p*18 + 2n + lane holds the (plane, tap) combo CONV2_PAIRS[n][lane]
    # (plane 0 = hi half of w2, 1 = lo half), so the device lhsT is a
    # plain adjacent-pair slice.
    wcq = np.zeros(wc.shape, FP8)
    if PREC == 'fp8':
        for p in range(2):
            hilo = {}
            for t in range(9):
                hilo[t] = _hilo(wc[:, :, p * 18 + t, :])
            for n, pair in enumerate(CONV2_PAIRS):
                for lane, (plane, t) in enumerate(pair):
                    wcq[:, :, p * 18 + 2 * n + lane, :] = hilo[t][plane]
    return wfq, wcq, wb.astype(BF16)


def _pack_x(x, mask, ns, placements):
    xm = (np.asarray(x)[0] * _expand_mask_np(mask)).astype(np.float32)
    xmp = np.pad(xm, ((0, 0), (1, 1), (1, 1)))
    nslot = ns[0] + ns[1]
    nk = 4 if PREC == 'fp8' else 8
    xs = np.zeros((NCORES, nslot, 128, nk, NPX), FP8)
    for g, (r, c), core, slot, hf in placements:
        hs = slice(64 * hf, 64 * hf + 64)
        win = xmp[g * 256:(g + 1) * 256,
                  16 * r:16 * r + GRID, 16 * c:16 * c + GRID]
        wk = win.reshape(4, 64, NPX).transpose(1, 0, 2)
        if PREC == 'fp8':
            xs[core, slot, hs] = wk.astype(FP8)
        else:
            hi, lo = _hilo(wk)
            xs[core, slot, hs, 0:4] = hi
            xs[core, slot, hs, 4:8] = lo
    return xs.reshape(NCORES, nslot, 128, nk * NPX)


def _epilogue(x, results, ns, placements):
    r = np.array(np.asarray(x)[0], dtype=np.float32, copy=True)
    nslot = ns[0] + ns[1]
    _, obatches = _batches(nslot)
    dec = []
    for core in range(NCORES):
        sat = 0
        cols = []
        for b, sl in enumerate(obatches):
            o = results[core][f'out{b}']
            sat = max(sat, np.abs(o.astype(np.int32)).max())
            cols.append(o.reshape(128, len(sl), 1024))
        if sat >= 127:
            raise FloatingPointError(
                'int8 conv3 output saturated; OUT_RANGE too small')
        dec.append(np.concatenate(cols, axis=1).astype(np.float32)
                   / OUT_SCALE)
    for g, (rr, cc), core, slot, hf in placements:
        o = dec[core][:, slot]
        blk = o[64 * hf:64 * hf + 64].reshape(64, 4, P, P)
        for j in range(4):
            ch = g * 256 + 64 * j
            r[ch:ch + 64, 16 * rr:16 * rr + P, 16 * cc:16 * cc + P] \
                += blk[:, j]
    return np.maximum(r, 0.0)[None]


# ---------------------------------------------------------------------------
# numpy golden model of the device program
# ---------------------------------------------------------------------------
def _q(a):
    return a.astype(BF16).astype(np.float32)


# fp8 conv2: 18 (plane, tap) combos -> 9 DR pairs; within each pair the
# second element must have the larger window offset (positive AP stride)
CONV2_PAIRS = [((0, 0), (0, 1)), ((0, 2), (0, 3)), ((0, 4), (0, 5)),
               ((0, 6), (0, 7)), ((1, 0), (0, 8)), ((1, 1), (1, 2)),
               ((1, 3), (1, 4)), ((1, 5), (1, 6)), ((1, 7), (1, 8))]


def _golden_results(xs, wfq, wcq, wb, ns):
    nslot = ns[0] + ns[1]
    nk = 4 if PREC == 'fp8' else 8
    nbp = 4 if PREC == 'fp8' else 13
    _, obatches = _batches(nslot)
    out_all = []
    for core in range(xs.shape[0]):
        wff = wfq[core].astype(np.float32).transpose(1, 0, 2)
        wcf = wcq[core].astype(np.float32).transpose(1, 0, 2)
        wbf = wb[core].astype(np.float32).transpose(1, 0, 2)
        outs = np.zeros((nslot, 128, 1024), np.float32)
        for slot in range(nslot):
            p = 0 if slot < ns[0] else 1
            xt = xs[core, slot].astype(np.float32).reshape(128, nk, NPX)
            ps1 = np.zeros((128, NPX), np.float32)
            if PREC == 'fp8':
                for hl in range(2):
                    for mm in range(2):
                        for i in range(2):
                            idx = ((p * 2 + hl) * 2 + mm) * 2 + i
                            ps1 += wff[idx].T @ xt[:, 2 * mm + i]
                t1 = np.maximum(ps1, 0.0).astype(FP8).astype(np.float32)
            else:
                for hl_w, x0 in ((0, 0), (1, 0), (0, 4)):
                    for mm in range(2):
                        for i in range(2):
                            idx = ((p * 2 + hl_w) * 2 + mm) * 2 + i
                            ps1 += wff[idx].T @ xt[:, x0 + 2 * mm + i]
                t1 = _q(np.maximum(ps1, 0.0))
            t1 = t1.reshape(128, GRID, GRID)
            ps2 = np.zeros((128, INT), np.float32)

            def twin(t):
                dy, dx = t // 3 - 1, t % 3 - 1
                return t1[:, 1 + dy:17 + dy, 1 + dx:17 + dx] \
                    .reshape(128, INT)
            if PREC == 'fp8':
                for n, pair in enumerate(CONV2_PAIRS):
                    for lane, (plane, t) in enumerate(pair):
                        ps2 += wcf[p * 18 + 2 * n + lane].T @ twin(t)
            else:
                for t in range(9):
                    ps2 += wbf[p * nbp + t].T @ twin(t)
            t2 = _q(np.maximum(ps2, 0.0))
            for j in range(4):
                off = p * nbp + (9 if PREC == 'bf16' else 0) + j
                outs[slot, :, 256 * j:256 * (j + 1)] = wbf[off].T @ t2
        res = {}
        for b, sl in enumerate(obatches):
            ob = np.zeros((128, len(sl) * 1024), np.int8)
            for i, s in enumerate(sl):
                ob[:, 1024 * i:1024 * (i + 1)] = \
                    (outs[s] * OUT_SCALE).astype(np.int8)
            res[f'out{b}'] = ob
        out_all.append(res)
    return out_all


def golden(x, mask, w1, w2, w3):
    ns, roles, placements = _plan(mask)
    wfq, wcq, wb = _pack_weights(w1, w2, w3, roles)
    xs = _pack_x(x, mask, ns, placements)
    res = _golden_results(xs, wfq, wcq, wb, ns)
    return _epilogue(x, res, ns, placements)


# ---------------------------------------------------------------------------
# Bass program
# ---------------------------------------------------------------------------
_NC_CACHE = {}


def _build_nc(ns0, ns1):
    import copy as _copy
    import concourse.bacc as bacc
    import concourse.mybir as mybir
    from concourse.tile import TileContext

    dt = mybir.dt
    f32 = dt.float32
    bf16 = dt.bfloat16
    fp8 = dt.float8e4
    i8 = dt.int8
    Relu = mybir.ActivationFunctionType.Relu
    DR = mybir.MatmulPerfMode.DoubleRow
    nslot = ns0 + ns1
    nk = 4 if PREC == 'fp8' else 8
    nbp = 4 if PREC == 'fp8' else 13
    xbatches, obatches = _batches(nslot)
    t1dt = fp8 if PREC == 'fp8' else bf16

    nc = bacc.Bacc(None, target_bir_lowering=False)
    xs_d = nc.declare_dram_parameter('xs', [nslot, 128, nk * NPX], fp8,
                                     isOutput=False)
    wf_d = nc.declare_dram_parameter('wf', [128, 16 * 128], fp8,
                                     isOutput=False)
    if PREC == 'fp8':
        wc_d = nc.declare_dram_parameter('wc', [128, 36 * 128], fp8,
                                         isOutput=False)
    wb_d = nc.declare_dram_parameter('wb', [128, 2 * nbp * 128], bf16,
                                     isOutput=False)
    out_d = [nc.declare_dram_parameter(f'out{b}', [128, len(sl) * 1024], i8,
                                       isOutput=True)
             for b, sl in enumerate(obatches)]

    with TileContext(nc) as tc:
        with (
            tc.tile_pool(name='const', bufs=1) as cpool,
            tc.tile_pool(name='xin', bufs=len(xbatches)) as xpool,
            tc.tile_pool(name='t1p', bufs=3) as t1pool,
            tc.tile_pool(name='t2p', bufs=3) as t2pool,
            tc.tile_pool(name='outp', bufs=len(obatches)) as opool,
            tc.tile_pool(name='ps1', bufs=2, space='PSUM') as ps1pool,
            tc.tile_pool(name='ps2', bufs=2, space='PSUM') as ps2pool,
            tc.tile_pool(name='ps3', bufs=4, space='PSUM') as ps3pool,
        ):
            wu = cpool.tile([128, 64], bf16, tag='wu')
            nc.vector.memset(wu[:], 0.0)
            psw = ps1pool.tile([128, NPX], f32, tag='ps1', name='warm')
            for _ in range(16):
                nc.tensor.matmul(psw[0:64, 0:64], wu[:], wu[:],
                                 start=True, stop=True)

            wf_sb = cpool.tile([128, 16, 128], fp8, tag='wf')
            if PREC == 'fp8':
                wc_sb = cpool.tile([128, 36, 128], fp8, tag='wc')
            wb_sb = cpool.tile([128, 2 * nbp, 128], bf16, tag='wb')
            xb = [None] * len(xbatches)

            def xload(b):
                if b >= len(xbatches) or xb[b] is not None:
                    return
                sl = xbatches[b]
                xt = xpool.tile([128, len(sl) * nk, NPX], fp8, tag='x',
                                name=f'xb{b}')
                xb[b] = xt
                nc.sync.dma_start(
                    out=xt[:],
                    in_=xs_d[sl[0]:sl[0] + len(sl)].rearrange(
                        's q c -> q s c'))

            # x slots stream as singleton DMAs on SP/HWDGE; weight halves
            # ride the otherwise-idle Pool SWDGE path, split per wset so the
            # p=1 tiles (first needed at slot ns0) never gate the front.
            # Interleave so each slot's x lands just ahead of its conv1.
            nwfh = 8 * 128
            xload(0)
            nc.gpsimd.dma_start(out=wf_sb[:, 0:8, :], in_=wf_d[:, 0:nwfh])
            xload(1)
            if PREC == 'fp8':
                nc.gpsimd.dma_start(out=wc_sb[:, 0:18, :],
                                    in_=wc_d[:, 0:18 * 128])
            xload(2)
            nc.gpsimd.dma_start(out=wb_sb[:], in_=wb_d[:, :])
            xload(3)
            xload(4)
            nc.sync.dma_start(out=wf_sb[:, 8:16, :], in_=wf_d[:, nwfh:])
            xload(5)
            if PREC == 'fp8':
                nc.sync.dma_start(out=wc_sb[:, 18:36, :],
                                  in_=wc_d[:, 18 * 128:])
            for b in range(6, len(xbatches)):
                xload(b)

            slot2b = {}
            for b, sl in enumerate(xbatches):
                for i, s in enumerate(sl):
                    slot2b[s] = (b, i)
            oslot2b = {}
            for b, sl in enumerate(obatches):
                for i, s in enumerate(sl):
                    oslot2b[s] = (b, i)

            t1s = [None] * nslot
            t2s = [None] * nslot
            ots = [None] * len(obatches)

            def stage_a(slot):
                p = 0 if slot < ns0 else 1
                bx, ix = slot2b[slot]
                k0 = ix * nk
                ps1 = ps1pool.tile([128, NPX], f32, tag='ps1', name='c1ps')
                if PREC == 'fp8':
                    seq = ((0, 0), (0, 1), (1, 0), (1, 1))   # (hl, mm)
                    for n, (hl, mm) in enumerate(seq):
                        idx = ((p * 2 + hl) * 2 + mm) * 2
                        nc.tensor.matmul(
                            ps1[:], wf_sb[:, idx:idx + 2, :],
                            xb[bx][:, k0 + 2 * mm:k0 + 2 * mm + 2, :],
                            start=(n == 0), stop=(n == 3), perf_mode=DR)
                else:
                    seq = ((0, 0, 0), (0, 0, 1), (1, 0, 0), (1, 0, 1),
                           (0, 4, 0), (0, 4, 1))             # (hl_w, xk, mm)
                    for n, (hl_w, xk, mm) in enumerate(seq):
                        idx = ((p * 2 + hl_w) * 2 + mm) * 2
                        nc.tensor.matmul(
                            ps1[:], wf_sb[:, idx:idx + 2, :],
                            xb[bx][:, k0 + xk + 2 * mm:
                                   k0 + xk + 2 * mm + 2, :],
                            start=(n == 0), stop=(n == 5), perf_mode=DR)
                t1 = t1pool.tile([128, NPX], t1dt, tag='t1', name='t1')
                t1s[slot] = t1
                nc.scalar.activation(t1[:], ps1[:], Relu)

            def stage_b(slot):
                p = 0 if slot < ns0 else 1
                t1 = t1s[slot]
                t1v = t1.rearrange('q (a b) -> q a b', b=GRID)
                ps2 = ps2pool.tile([128, INT], f32, tag='ps2', name='c2ps')
                if PREC == 'fp8':
                    offs = [(t // 3) * GRID + (t % 3) for t in range(9)]
                    for n, ((_, ta), (_, tb)) in enumerate(CONV2_PAIRS):
                        oa, ob_ = offs[ta], offs[tb]
                        rhs = _copy.deepcopy(t1[:, oa:])
                        rhs.ap = rhs.ap[:1] + [[ob_ - oa, 2], [GRID, P],
                                               [1, P]]
                        base = p * 18 + 2 * n
                        nc.tensor.matmul(ps2[:], wc_sb[:, base:base + 2, :],
                                         rhs, start=(n == 0), stop=(n == 8),
                                         perf_mode=DR)
                else:
                    for t in range(9):
                        dy, dx = t // 3 - 1, t % 3 - 1
                        win = t1v[:, 1 + dy:17 + dy, 1 + dx:17 + dx]
                        nc.tensor.matmul(ps2[:], wb_sb[:, p * nbp + t, :],
                                         win, start=(t == 0), stop=(t == 8))
                t2 = t2pool.tile([128, INT], bf16, tag='t2', name='t2')
                t2s[slot] = t2
                nc.vector.tensor_scalar_max(t2[:], ps2[:], 0.0)

            def stage_c(slot):
                p = 0 if slot < ns0 else 1
                ob, io = oslot2b[slot]
                if io == 0:
                    ots[ob] = opool.tile([128, len(obatches[ob]), 1024], i8,
                                         tag='out', name=f'ot{ob}')
                ot = ots[ob]
                joff = p * nbp + (9 if PREC == 'bf16' else 0)
                for half in range(2):
                    ps3 = ps3pool.tile([128, 512], f32, tag='ps3',
                                       name='c3ps')
                    for jj in range(2):
                        j = 2 * half + jj
                        nc.tensor.matmul(ps3[:, 256 * jj:256 * (jj + 1)],
                                         wb_sb[:, joff + j, :],
                                         t2s[slot][:],
                                         start=True, stop=True)
                    dst = ot[:, io, 512 * half:512 * (half + 1)]
                    # h1 converts on ACT, h0 on DVE: balances the two
                    # engines and lets every slot's drain run in parallel
                    if half == 1:
                        nc.scalar.activation(
                            dst, ps3[:], mybir.ActivationFunctionType.Copy,
                            scale=float(OUT_SCALE))
                    else:
                        nc.vector.tensor_scalar_mul(dst, ps3[:], OUT_SCALE)
                if io == len(obatches[ob]) - 1:
                    nc.sync.dma_start(out=out_d[ob][:, :], in_=ot[:])

            # software-pipelined issue order: A(s) runs two slots ahead of
            # C(s) so the PE never waits on the ACT relus in the slot chain
            for i in range(nslot + 2):
                if i < nslot:
                    stage_a(i)
                if 1 <= i <= nslot:
                    stage_b(i - 1)
                if i >= 2:
                    stage_c(i - 2)
    nc.finalize()
    return nc


def _get_nc(ns=(4, 4)):
    key = (tuple(ns), PREC)
    if key not in _NC_CACHE:
        _NC_CACHE[key] = _build_nc(*ns)
    _NC_CACHE['nc'] = _NC_CACHE[key]
    return _NC_CACHE[key]


def kernel(x, mask, w1, w2, w3):
    from concourse.bass_utils import run_bass_kernel_spmd

    ns, roles, placements = _plan(mask)
    wfq, wcq, wb = _pack_weights(w1, w2, w3, roles)
    xs = _pack_x(x, mask, ns, placements)
    nc = _get_nc(ns)
    in_maps = []
    for h in range(NCORES):
        im = {'xs': xs[h], 'wf': wfq[h].reshape(128, -1),
              'wb': wb[h].reshape(128, -1)}
        if PREC == 'fp8':
            im['wc'] = wcq[h].reshape(128, -1)
        in_maps.append(im)
    res = run_bass_kernel_spmd(nc, in_maps, list(range(NCORES))).results
    return _epilogue(x, res, ns, placements)
